# revision 42
# baseline (speedup 1.0000x reference)
"""Trainium2 Bass kernel for nn_Decoder_40338332844507.

Computes logits = einsum('btc,wpc->bptw', q, W) + b.T[None,:,None,:]
with q [32, 2048, 256] f32, W [49, 32, 256] f32, b [49, 32] f32,
output [32, 32, 2048, 49] f32.

Strategy: data-parallel over batch across 8 NeuronCores (4 batches per
core). Per core, for each 128-token tile the TensorEngine computes
out[t, n] = qT_tile.T @ Wr in bf16 (n = p*49+w flattened, split into
column groups pg0=[0,772) and pg1=[772,1568)). Tokens are tiled
contiguously (t = tl*128 + tp) so the stationary operand is a
contiguous [128,128] slice, which enables the PE's fast-weight-load
path (stride-16 slices measured LDWEIGHTS at 144ns vs ~85ns here).
The eviction PSUM->SBUF is split across two engines so it never paces
the PE (in the 169us baseline a DVE-only tensor_tensor eviction was
129us busy and throttled the matmul stream through PSUM backpressure):
  pg0: DVE tensor_tensor  psum + bias_f32 -> bf16 SBUF  (~960ns/tile)
  pg1: ACT copy           psum -> bf16 SBUF             (~925ns/tile)
(walrus rejects every gpsimd ALU op, so the bias for pg1's columns is
added by the host during the gather — a masked broadcast add.)
Output is stored as bf16 (rel-err budget 2e-2 >> bf16's ~0.4%) in a
fully sequential per-tile DRAM layout [b, tp, l, n] so each store
descriptor covers >=11KB contiguous lines; the host reassembles to
[B, P, T, W] and upcasts to f32 during the gather. This halves store
traffic (51.4 -> 25.7 MB/core), which was the baseline's critical path
(it ended with a 32us pure store-drain tail at ~334 GB/s).
Scheduling details that each bought measurable time: loads are ordered
critical-prefix-first on the two HWDGE rings (ring FIFO = priority, so
later batches can't steal HBM bandwidth from the first batch's q/wr),
bias rides the otherwise-idle SWDGE ring, the scalar engine's lazy
1.28us ACT_TABLE_LOAD is hoisted into the load phase, and 9 warmup
matmuls bridge the runtime preamble so the PE HAM clock-gate is at
2.4 GHz (not the cold 1.2) when real work arrives.
"""

import json
import sys
import numpy as np
from contextlib import ExitStack

if "/opt/trn_rl_repo" not in sys.path:
    sys.path.insert(0, "/opt/trn_rl_repo")

import concourse.bass as bass
import concourse.tile as tile
from concourse import mybir
from concourse.bass_utils import run_bass_kernel_spmd

B, T, C = 32, 2048, 256
P, WW = 32, 49
N = P * WW  # 1568
NPG0 = 772  # DVE-evicted columns (bias fused on device)
NPG1 = N - NPG0  # 796 ACT-evicted columns (bias added on host)
N_CORES = 8
B_LOC = B // N_CORES  # 4 batches per core
TL = 16  # token interleave: t = tp*16 + tl


def _patch_split_sync_waits():
    """The walrus build on this image accepts at most ONE sync-wait per
    instruction ("Too many sync wait commands" otherwise). Tile emits
    instructions with several waits. Post-process the serialized BIR:
    hoist all but the last wait of each instruction onto 1-wait NoOps
    inserted immediately before it on the same engine (engines execute
    their instruction stream in order, so the semantics are identical)."""
    if getattr(bass.Bass, "_split_waits_patched", False):
        return
    orig = bass.Bass.to_json_bytes

    def to_json_bytes(self):
        m = json.loads(orig(self))
        ctr = 0
        for f in m.get("functions", []):
            for bb in f.get("blocks", []):
                out = []
                for inst in bb.get("instructions", []):
                    si = inst.get("sync_info")
                    if si:
                        waits = si.get("on_wait") or []
                        if len(waits) > 1:
                            for wt in waits[:-1]:
                                ctr += 1
                                nop = {
                                    "engine": inst["engine"],
                                    "ins": [],
                                    "outs": [],
                                    "name": f"I-npw{ctr}",
                                    "opcode": "NoOp",
                                    "sync_info": {"on_wait": [wt], "on_update": []},
                                }
                                if inst.get("debug") is not None:
                                    nop["debug"] = inst["debug"]
                                out.append(nop)
                            si["on_wait"] = waits[-1:]
                    out.append(inst)
                bb["instructions"] = out
        return json.dumps(m).encode()

    bass.Bass.to_json_bytes = to_json_bytes
    bass.Bass._split_waits_patched = True


# NOTE: forcing --enable-ldw-opt=true in the walrus invocation was tried
# and CRASHES walrus codegen (visitInstLdweights, CoreV3GenImpl.cpp:694)
# — the flag is hardcoded off in bass_utils for a reason.


def build_bass():
    _patch_split_sync_waits()
    nc = bass.Bass("TRN2", target_bir_lowering=False, debug=False)
    f32 = mybir.dt.float32
    bf16 = mybir.dt.bfloat16

    qt = nc.dram_tensor("qt", [B_LOC, C, T], bf16, kind="ExternalInput")
    wr = nc.dram_tensor("wr", [C, N], bf16, kind="ExternalInput")
    # boot[k] = host-packed [wr[k][:,0:512] | q[b0][k][:,0:512]]: the
    # critical prefix as ONE contiguous 262KB transfer per ring (two
    # separate 131KB loads with 1KB/partition lines ran at ~40% DMA
    # efficiency and delayed the first full-rate matmul to ~14.5us)
    boot = [
        nc.dram_tensor(f"boot{k}", [128, 1024], bf16, kind="ExternalInput")
        for k in range(2)
    ]
    bias0 = nc.dram_tensor("bias0", [128, NPG0], f32, kind="ExternalInput")
    o0 = nc.dram_tensor("o0", [B_LOC, 128, TL, NPG0], bf16, kind="ExternalOutput")
    o1 = nc.dram_tensor("o1", [B_LOC, 128, TL, NPG1], bf16, kind="ExternalOutput")

    with tile.TileContext(nc) as tc:
        with ExitStack() as ctx:
            consts = ctx.enter_context(tc.tile_pool(name="consts", bufs=1))
            opool0 = ctx.enter_context(tc.tile_pool(name="opool0", bufs=2))
            opool1 = ctx.enter_context(tc.tile_pool(name="opool1", bufs=2))
            psum = ctx.enter_context(tc.tile_pool(name="psum", bufs=4, space="PSUM"))

            wr_sb = [
                consts.tile([128, N], bf16, tag=f"wr{k}", name=f"wr{k}")
                for k in range(2)
            ]
            # q for all 4 batches is prefetched up front (32KB/partition);
            # b0 goes on the fast HWDGE rings right after wr so compute can
            # start ~3us in, the rest streams on the gpsimd (SWDGE) ring.
            q_sb = [
                [
                    consts.tile([128, T], bf16, tag=f"q{b}_{k}", name=f"q{b}_{k}")
                    for k in range(2)
                ]
                for b in range(B_LOC)
            ]
            # Critical-prefix load order on the two HWDGE rings (ring FIFO
            # = priority): the packed boot blob first, then the wr/q[b0]
            # tails, then the later batches. The first matmuls read the
            # boot tile directly; wr_sb[:, 0:512] and q_sb[0][:, 0:512]
            # are never written or read.
            boot_sb = [
                consts.tile([128, 1024], bf16, tag=f"boot{k}", name=f"boot{k}_sb")
                for k in range(2)
            ]
            scratch = consts.tile([128, 16], bf16, tag="scratch", name="scratch")
            nc.sync.dma_start(boot_sb[0][:], boot[0].ap()[:, :])
            nc.scalar.dma_start(boot_sb[1][:], boot[1].ap()[:, :])
            nc.sync.dma_start(wr_sb[0][:, 512:N], wr.ap()[0:128, 512:N])
            nc.scalar.dma_start(wr_sb[1][:, 512:N], wr.ap()[128:256, 512:N])
            nc.sync.dma_start(
                q_sb[0][0][:, 512:T], qt.ap()[0, 0:128, 512:T]
            )
            nc.scalar.dma_start(
                q_sb[0][1][:, 512:T], qt.ap()[0, 128:256, 512:T]
            )

            # bias0 rides the otherwise-idle gpsimd (SWDGE) ring: its
            # first packet lands ~11us and it must be resident before the
            # first DVE eviction (~15us). Queued behind q01 it arrived at
            # ~18us and stalled the whole PSUM pipeline (v6 post-mortem).
            bias0_sb = consts.tile([128, NPG0], f32, tag="bias0", name="bias0_sb")
            nc.gpsimd.dma_start(bias0_sb[:], bias0.ap()[:, :])
            # q[b1..b3] go on the gpsimd ring, GATED behind q[b0]: a
            # tiny DVE add reads both the gated DMA's dest cell and a
            # q[b0] cell, so the gated doorbell waits via a WAR dep
            # (Tile elides WAW into a full-overwrite DMA, so a plain
            # write-gate does NOT work — measured in v5b). Keeping all
            # 3MB of later-batch loads off the store rings leaves them
            # carrying only the 0.92MB critical prefix each, so the
            # store stream starts as soon as the first evictions land
            # and the end drain tail shrinks. q[b1] is needed at ~36us
            # and the gated SWDGE ring delivers it by ~20us.
            nc.vector.memset(q_sb[1][0][0:1, 0:1], 0)
            nc.vector.tensor_add(
                scratch[0:1, 4:5],
                q_sb[1][0][0:1, 0:1],
                q_sb[0][0][0:1, 512:513],
            )
            for b in range(1, B_LOC):
                nc.gpsimd.dma_start(q_sb[b][0][:], qt.ap()[b, 0:128, :])
                nc.gpsimd.dma_start(q_sb[b][1][:], qt.ap()[b, 128:256, :])

            # PE warmup: dummy matmuls bridge from the end of the runtime
            # preamble (~8.3us) until the critical q[b0]/wr prefix lands
            # (~10.8us), lifting the HAM clock gate toward 2.4 GHz.
            warm_sb = consts.tile([128, 512], bf16, tag="warm", name="warm_sb")
            nc.vector.memset(warm_sb[:], 0)
            # Hoist the scalar engine's lazy ACT_TABLE_LOAD (1.28us) into
            # the load phase — it otherwise fires right before the first
            # pg1 eviction and stalls the PSUM pipeline.
            nc.vector.memset(scratch[:, 0:1], 0)
            nc.scalar.copy(scratch[:, 1:2], scratch[:, 0:1])
            for i in range(4):
                wpt = psum.tile([128, 1024], f32, tag="pt", name=f"warm{i}")
                nc.tensor.matmul(
                    wpt[:, 0:512], warm_sb[:, 0:128], warm_sb[:],
                    start=True, stop=True,
                )

            groups = (
                (NPG0, 0, o0, nc.sync),
                (NPG1, NPG0, o1, nc.scalar),
            )
            for b in range(B_LOC):
                # token block tl covers t = tl*128 .. tl*128+127, so the
                # stationary operand q_sb[:, tl*128:(tl+1)*128] is a
                # CONTIGUOUS [128,128] slice — eligible for the PE's fast
                # weight load path (strided weights defeat FWL). b0's
                # first 4 token blocks live in the packed boot tile.
                q_v = [
                    [
                        boot_sb[k][:, 512 + tl * 128 : 512 + (tl + 1) * 128]
                        if (b == 0 and tl < 4)
                        else q_sb[b][k][:, tl * 128 : (tl + 1) * 128]
                        for tl in range(TL)
                    ]
                    for k in range(2)
                ]
                oh = [
                    opool.tile([128, TL * g[0]], bf16, tag=f"oh{pg}", name=f"oh{b}_{pg}")
                    for pg, (opool, g) in enumerate(zip((opool0, opool1), groups))
                ]
                # half-size stores (stores queue behind the loads on the
                # rings, so extra-early small stores buy nothing); taper
                # at the very end of the last batch to shrink the final
                # store-drain tail
                if b == B_LOC - 1:
                    splits = (4, 8, 12, 14, 16)
                else:
                    splits = (8, 16)
                l_done = 0
                # tl-outer / pg-inner: the DVE (pg0) and ACT (pg1)
                # evictions alternate so they drain PSUM concurrently;
                # k-outer matmuls halve the LDWEIGHTS count.
                for tl in range(TL):
                    for pg, (npg, cbase, odram, seng) in enumerate(groups):
                        pt = psum.tile(
                            [128, 1024], f32, tag="pt", name=f"pt{b}_{pg}_{tl}"
                        )
                        for k, (start, stop) in enumerate(((True, False), (False, True))):
                            for n0, n1 in ((0, 512), (512, npg)):
                                rhs = (
                                    boot_sb[k][:, cbase + n0 : cbase + n1]
                                    if cbase + n1 <= 512
                                    else wr_sb[k][:, cbase + n0 : cbase + n1]
                                )
                                nc.tensor.matmul(
                                    pt[:, n0:n1], q_v[k][tl], rhs,
                                    start=start, stop=stop,
                                )
                        # single eviction per tile: splitting it per chunk
                        # was tried and REGRESSED (the doubled sem traffic
                        # slowed the matmul cadence 196 -> 209 ns/MM)
                        dst = oh[pg][:, bass.ds(tl * npg, npg)]
                        if pg == 0:
                            nc.vector.tensor_add(dst, pt[:, 0:npg], bias0_sb[:])
                        else:
                            nc.scalar.copy(dst, pt[:, 0:npg])
                    if tl + 1 in splits:
                        for pg, (npg, cbase, odram, seng) in enumerate(groups):
                            dram = (
                                odram.ap()[b, :, l_done : tl + 1, :]
                                .rearrange("t l n -> t (l n)")
                            )
                            seng.dma_start(
                                dram,
                                oh[pg][:, bass.ds(l_done * npg, (tl + 1 - l_done) * npg)],
                            )
                        l_done = tl + 1
    return nc


_NC_CACHE = None


def _get_nc():
    global _NC_CACHE
    if _NC_CACHE is None:
        _NC_CACHE = build_bass()
    return _NC_CACHE


def prep_in_maps(q, W, b):
    """Host-side layout prep: weight packing + activation transpose +
    bf16 cast + bias broadcast tile. Returns per-core input maps."""
    import ml_dtypes

    bf = ml_dtypes.bfloat16
    Wt = np.asarray(W, dtype=np.float32)
    bias = np.asarray(b, dtype=np.float32)
    q = np.asarray(q, dtype=np.float32)

    qt = np.ascontiguousarray(q.transpose(0, 2, 1).astype(bf))  # [B, C, T]
    wrm = np.ascontiguousarray(Wt.transpose(2, 1, 0).reshape(C, N).astype(bf))
    bias_flat = bias.T.reshape(N)  # n = p*49 + w
    b0 = np.ascontiguousarray(
        np.broadcast_to(bias_flat[:NPG0].reshape(1, NPG0), (128, NPG0)).astype(
            np.float32
        )
    )
    maps = []
    for c in range(N_CORES):
        qc = qt[c * B_LOC : (c + 1) * B_LOC]
        boots = [
            np.ascontiguousarray(
                np.concatenate(
                    [wrm[k * 128 : (k + 1) * 128, 0:512], qc[0, k * 128 : (k + 1) * 128, 0:512]],
                    axis=1,
                )
            )
            for k in range(2)
        ]
        maps.append(
            {
                "qt": qc,
                "wr": wrm,
                "boot0": boots[0],
                "boot1": boots[1],
                "bias0": b0,
            }
        )
    return maps


def _host_bias(b):
    """Bias for the ACT-evicted columns (n >= NPG0); zero elsewhere
    (those got their bias on-device, fused into the DVE eviction)."""
    bias_flat = np.asarray(b, dtype=np.float32).T.reshape(N).copy()
    bias_flat[:NPG0] = 0.0
    return bias_flat.reshape(1, P, 1, WW)


def postprocess(core_outs, b):
    """Reassemble per-core ([B_LOC,128,TL,NPG0], [B_LOC,128,TL,NPG1])
    bf16 device tiles into the full [B, P, T, W] f32 output."""
    hb = _host_bias(b)
    parts = []
    for a0, a1 in core_outs:
        a = np.concatenate([np.asarray(a0), np.asarray(a1)], axis=3)
        # [b, tp, l, n] with t = l*128 + tp -> [b, p, l, tp, w] -> [b, p, t, w]
        a = (
            a.reshape(B_LOC, 128, TL, P, WW)
            .transpose(0, 3, 2, 1, 4)
            .reshape(B_LOC, P, T, WW)
            .astype(np.float32)
        )
        parts.append(a + hb)
    return np.concatenate(parts, axis=0)


def kernel(q, W, b):
    nc = _get_nc()
    in_maps = prep_in_maps(q, W, b)
    res = run_bass_kernel_spmd(nc, in_maps, core_ids=list(range(N_CORES)))
    return postprocess(
        [(res.results[c]["o0"], res.results[c]["o1"]) for c in range(N_CORES)], b
    )
